# revision 1
# baseline (speedup 1.0000x reference)
"""AttentionPerLabelWordLevel Trainium2 kernel (8-core SPMD, batch-sharded).

Reference computation (per batch b):
  h = tanh(x @ W.T + b)                      # [T, H]
  logits = h @ C.T                           # [S, L, C]
  m = max_L(logits)                          # [S, 1, C]
  attn = softmax_C(logits - m)               # [S, L, C]
  out[s, c, :] = sum_l attn[s, l, c] * x[s, l, :]   # [S, C, H]

Shapes: B=32, T=2500 (S=100 sentences x L=25 words), H=512, C=50.
Sharding: data-parallel over batch, 4 batches per core.

Per-core strategy (x, W, C are pre-cast to float16 on the host — an
11-bit-mantissa format that runs 1 cycle/row on the PE with fast
pipelined weight loads and halves the load DMA volume):
  - x is DMA'd once per 16-sentence wave into f16 "packed" tiles
    [128, 512] holding 4 sentences at partition offsets 0/32/64/96
    (25 words + 7 pad rows each).
  - x^T and e^T come from f16 PE transposes (1 cycle/row) into
    full-bank f16 PSUM tiles, copied back with few wide DVE/ACT ops.
  - h^T, logits, e live on a padded t-axis (32 slots/sentence).
  - Softmax normalization is a batched per-word scale of the
    attention weights (single tensor_tensor op per wave).
  - The output einsum runs as f16 matmuls packed 4x along K (row
    groups) and 2x along M (col groups).
  - DMA traffic is spread over three initiators: Sync HWDGE,
    Scalar HWDGE, and GpSimd SWDGE.
"""

import numpy as np

import concourse.bacc as bacc
import concourse.bass as bass
import concourse.tile as tile
from concourse import mybir
from concourse.bass_utils import run_bass_kernel_spmd
from concourse.masks import make_identity

F32 = mybir.dt.float32
F16 = mybir.dt.float16
AX = mybir.AxisListType
AF = mybir.ActivationFunctionType

N_CORES = 8
B = 32
S = 100          # sentences per batch
L = 25           # words per sentence
C = 50           # classes
H = 512          # hidden
B_LOC = B // N_CORES          # batches per core
WAVE_S = 16                   # sentences per wave (4 packed tiles)
N_WAVES = 7                   # 6 full waves + 1 final wave (4 sentences)

_CACHE = {}
LAST_RESULT = None


def build_nc():
    nc = bacc.Bacc(trn_type="TRN2", target_bir_lowering=False, debug=False,
                   num_swdge_queues=2)
    x_d = nc.declare_dram_parameter("input_tensor", [B_LOC, S * L, H], F16, isOutput=False)
    w_d = nc.declare_dram_parameter("W", [H, H], F16, isOutput=False)
    b_d = nc.declare_dram_parameter("b", [H], F32, isOutput=False)
    c_d = nc.declare_dram_parameter("context_vector", [C, H], F16, isOutput=False)
    o_d = nc.declare_dram_parameter("out", [B_LOC, S, C, H], F32, isOutput=True)

    q_load = [nc.sync, nc.scalar]
    q_store = [nc.gpsimd, nc.sync, nc.scalar]

    with tile.TileContext(nc) as tc:
        with tc.tile_pool(name="sb", bufs=1) as sb, \
             tc.tile_pool(name="consts", bufs=1) as consts, \
             tc.tile_pool(name="ps", bufs=1, space="PSUM") as ps:

            # ---------------- one-time setup ----------------
            ident_f = consts.tile([128, 128], F32)
            make_identity(nc, ident_f)
            ident_h = consts.tile([128, 128], F16)
            nc.vector.tensor_copy(ident_h, ident_f)

            b_sb = consts.tile([128, 4], F32)
            nc.sync.dma_start(out=b_sb, in_=b_d.rearrange("(k p) -> p k", p=128))

            # W^T tiles: W_T[i] is [i-part 128, o 512] (f16), via xbar
            w_t = []
            for i in range(4):
                wt = consts.tile([128, 512], F16, name=f"w_t{i}")
                w_t.append(wt)
            for o in range(4):
                wh = consts.tile([128, 512], F16, name=f"w_nat{o}")
                nc.sync.dma_start(out=wh, in_=w_d[o * 128:(o + 1) * 128, :])
                for i in range(4):
                    nc.scalar.dma_start_transpose(
                        w_t[i][:, o * 128:(o + 1) * 128],
                        wh[:, i * 128:(i + 1) * 128],
                    )

            # C^T tile: [o-part 128, o_chunk 4, c 64] (f16), via xbar
            c_h = consts.tile([64, 512], F16)
            nc.sync.dma_start(out=c_h[:C, :], in_=c_d[:, :])
            c_t = consts.tile([128, 4, 64], F16)
            for o in range(4):
                nc.scalar.dma_start_transpose(
                    c_t[:, o, :], c_h[:, o * 128:(o + 1) * 128]
                )

            # ---------------- main loop (software-pipelined) ----------------
            def emit_front(bi, wv):
                s0 = wv * WAVE_S
                ns = min(WAVE_S, S - s0)      # 16 or 4
                G = ns // 4                   # packed tiles (4 or 1)
                W_COLS = 32 * ns              # padded t-cols (512 or 128)

                # -- load packed f16 x: one DMA per word-row-block jj --
                xp_all = sb.tile([128, 2088], F16, tag="xp", bufs=3,
                                 name=f"xp{bi}_{wv}")
                for jj in range(4):
                    dvw = xp_all[32 * jj:32 * jj + L, :]
                    dst = bass.AP(tensor=xp_all.tensor, offset=dvw.offset,
                                  ap=[dvw.ap[0], [520, G], [1, 512]])
                    svw = x_d[bi, (s0 + jj) * L:(s0 + jj) * L + 1, :]
                    srcv = bass.AP(tensor=svw.tensor, offset=svw.offset,
                                   ap=[[512, L], [4 * L * 512, G], [1, 512]])
                    q_load[jj % 2].dma_start(out=dst, in_=srcv)

                def xp(g):
                    return xp_all[:, 520 * g:520 * g + 512]

                # -- x^T via f16 PE transposes into 2 full psum banks --
                xt_sb = []
                for half in range(2):
                    pxt = ps.tile([128, 1024], F16, tag="xt", bufs=2,
                                  name=f"pxt{bi}_{wv}_{half}")
                    for il in range(2):
                        i = 2 * half + il
                        for g in range(G):
                            nc.tensor.transpose(
                                pxt[:, 512 * il + 128 * g:
                                512 * il + 128 * (g + 1)],
                                xp(g)[:, i * 128:(i + 1) * 128],
                                ident_h,
                            )
                    xs = sb.tile([128, 1024], F16, tag="xt_sb", bufs=4,
                                 name=f"xt_sb{bi}_{wv}_{half}")
                    nc.vector.tensor_copy(xs.bitcast(F32), pxt.bitcast(F32))
                    xt_sb.append(xs)


                return bi, wv, xp_all, xt_sb

            def emit_back(state):
                bi, wv, xp_all, xt_sb = state
                s0 = wv * WAVE_S
                ns = min(WAVE_S, S - s0)
                G = ns // 4
                W_COLS = 32 * ns

                def xp(g):
                    return xp_all[:, 520 * g:520 * g + 512]

                def xt_rhs(i):
                    return xt_sb[i // 2][:, 512 * (i % 2):
                                         512 * (i % 2) + W_COLS]

                # -- step 1: h^T[o] = tanh(W @ x^T + b), f16 --
                h = []
                for o in range(4):
                    ph = ps.tile([128, W_COLS], F32, tag="ph", bufs=2,
                                 name=f"ph{bi}_{wv}_{o}")
                    for i in range(4):
                        nc.tensor.matmul(
                            ph,
                            w_t[i][:, o * 128:(o + 1) * 128],
                            xt_rhs(i),
                            start=(i == 0), stop=(i == 3),
                        )
                    ht = sb.tile([128, 512], F16, tag="h", bufs=8,
                                 name=f"h{bi}_{wv}_{o}")
                    nc.scalar.activation(
                        out=ht[:, :W_COLS], in_=ph,
                        func=AF.Tanh, bias=b_sb[:, o:o + 1], scale=1.0,
                    )
                    h.append(ht)

                # -- step 2: logits[c, t] (accumulate over o) --
                pl = ps.tile([C, W_COLS], F32, tag="xt", bufs=2,
                             name=f"pl{bi}_{wv}")
                for o in range(4):
                    nc.tensor.matmul(
                        pl, c_t[:, o, :C], h[o][:, :W_COLS],
                        start=(o == 0), stop=(o == 3),
                    )

                # -- m = max over words (strided view skips pad cols) --
                m = sb.tile([C, WAVE_S], F32, tag="m", bufs=3,
                            name=f"m{bi}_{wv}")
                pl_v = bass.AP(tensor=pl.tensor, offset=pl.offset,
                               ap=[pl.ap[0], [32, ns], [1, L]])
                nc.vector.reduce_max(out=m[:, :ns], in_=pl_v, axis=AX.X)

                # -- e = exp(logits - m) (strided, padded layout kept) --
                epre = sb.tile([C, 512], F16, tag="epre", bufs=3,
                               name=f"epre{bi}_{wv}")
                e_sb = sb.tile([C, 512], F16, tag="e", bufs=3,
                               name=f"e{bi}_{wv}")
                ep_v = bass.AP(tensor=epre.tensor, offset=epre.offset,
                               ap=[epre.ap[0], [32, ns], [1, L]])
                e_v = bass.AP(tensor=e_sb.tensor, offset=e_sb.offset,
                              ap=[e_sb.ap[0], [32, ns], [1, L]])
                m_v = bass.AP(tensor=m.tensor, offset=m.offset,
                              ap=[m.ap[0], [1, ns], [0, L]])
                nc.vector.tensor_sub(ep_v, pl_v, m_v)
                nc.scalar.activation(out=e_v, in_=ep_v, func=AF.Exp)

                # -- e^T via f16 PE transposes -> one merged attn tile --
                pet = ps.tile([128, 256], F16, tag="xt", bufs=2,
                              name=f"pet{bi}_{wv}")
                for g in range(G):
                    nc.tensor.transpose(
                        pet[:, 64 * g:64 * g + C],
                        e_sb[:, 128 * g:128 * (g + 1)],
                        ident_h[:C, :C],
                    )
                attn = sb.tile([128, 256], F16, tag="attn", bufs=4,
                               name=f"attn{bi}_{wv}")
                nc.vector.tensor_copy(attn.bitcast(F32), pet.bitcast(F32))

                # batched normalization: z[g] = 1/sum_c, attn *= z
                att_v = bass.AP(tensor=attn.tensor, offset=attn.offset,
                                ap=[attn.ap[0], [64, G], [1, C]])
                z = sb.tile([128, 4], F32, tag="z", bufs=4,
                            name=f"z{bi}_{wv}")
                nc.vector.reduce_sum(out=z[:, :G], in_=att_v, axis=AX.X)
                nc.vector.reciprocal(out=z[:, :G], in_=z[:, :G])
                z_v = bass.AP(tensor=z.tensor, offset=z.offset,
                              ap=[z.ap[0], [1, G], [0, C]])
                nc.vector.tensor_mul(att_v, att_v, z_v)

                # -- step 5: out[c, o] per sentence; 4xK 2xM packed f16 --
                n_pairs = max(1, G // 2)
                si = 0
                for pi in range(n_pairs):
                    gl_count = 2 if G >= 2 else 1
                    osb = sb.tile([128, 2088], F32, tag="osb", bufs=6,
                                  name=f"osb{bi}_{wv}_{pi}")
                    for jj in range(4):
                        po = ps.tile([128, 512], F32, tag=f"po{jj % 2}",
                                 bufs=2, name=f"po{bi}_{wv}_{pi}_{jj}")
                        for gl in range(gl_count):
                            g = 2 * pi + gl
                            nc.tensor.matmul(
                                po[64 * gl:64 * gl + C, :],
                                attn[32 * jj:32 * jj + L,
                                 64 * g:64 * g + C],
                                xp(g)[32 * jj:32 * jj + L, :],
                                start=True, stop=True,
                                tile_position=(32 * jj, 64 * gl),
                            )
                        ncols = 64 * (gl_count - 1) + C
                        dstc = osb[:ncols, 520 * jj:520 * jj + 512]
                        if jj % 2 == 0:
                            nc.vector.tensor_copy(dstc, po[:ncols, :])
                        else:
                            nc.scalar.copy(dstc, po[:ncols, :])
                    for gl in range(gl_count):
                        sbase = s0 + 8 * pi + 4 * gl
                        ovw = osb[64 * gl:64 * gl + C, :]
                        srcv = bass.AP(tensor=osb.tensor, offset=ovw.offset,
                                   ap=[ovw.ap[0], [520, 4], [1, 512]])
                        dvw = o_d[bi, sbase:sbase + 1]
                        dst = bass.AP(tensor=dvw.tensor, offset=dvw.offset,
                                  ap=[[512, C], [C * 512, 4], [1, 512]])
                        q_store[si % 3].dma_start(out=dst, in_=srcv)
                        si += 1

            pending = []
            for bi in range(B_LOC):
                for wv in range(N_WAVES):
                    pending.append(emit_front(bi, wv))
                    if len(pending) >= 1:
                        emit_back(pending.pop(0))
            while pending:
                emit_back(pending.pop(0))

    nc.compile()
    return nc


def kernel(**inputs):
    global LAST_RESULT
    if "nc" not in _CACHE:
        _CACHE["nc"] = build_nc()
    nc = _CACHE["nc"]

    x = np.asarray(inputs["input_tensor"], dtype=np.float32).astype(np.float16)
    w = np.asarray(inputs["W"], dtype=np.float32).astype(np.float16)
    bb = np.ascontiguousarray(inputs["b"], dtype=np.float32)
    cv = np.asarray(inputs["context_vector"], dtype=np.float32).astype(np.float16)

    in_maps = [
        {
            "input_tensor": np.ascontiguousarray(x[ci * B_LOC:(ci + 1) * B_LOC]),
            "W": w,
            "b": bb,
            "context_vector": cv,
        }
        for ci in range(N_CORES)
    ]
    res = run_bass_kernel_spmd(nc, in_maps, core_ids=list(range(N_CORES)))
    LAST_RESULT = res
    out = np.empty((B, S, C, H), dtype=np.float32)
    for ci in range(N_CORES):
        out[ci * B_LOC:(ci + 1) * B_LOC] = res.results[ci]["out"]
    return out



# revision 2
# speedup vs baseline: 1.3230x; 1.3230x over previous
"""AttentionPerLabelWordLevel Trainium2 kernel (8-core SPMD, batch-sharded).

Reference computation (per batch b):
  h = tanh(x @ W.T + b)                      # [T, H]
  logits = h @ C.T                           # [S, L, C]
  m = max_L(logits)                          # [S, 1, C]
  attn = softmax_C(logits - m)               # [S, L, C]
  out[s, c, :] = sum_l attn[s, l, c] * x[s, l, :]   # [S, C, H]

Shapes: B=32, T=2500 (S=100 sentences x L=25 words), H=512, C=50.
Sharding: data-parallel over batch, 4 batches per core.

v2 design (vs the transpose-on-device baseline):
  - x^T is pre-transposed on the HOST and DMA'd directly ([B_loc, H, T]
    f16, 5 KB contiguous lines) - no PE transposes, no PSUM->SBUF copies
    for x^T, and step 1/2 run on a DENSE t axis (no 25->32 padding).
  - x natural (einsum rhs) is pre-packed on the HOST into the exact
    SBUF image: waves of 16 sentences, 4 groups of 4 sentences at
    32-partition offsets ([B_loc, 7, 128, 2048] f16, 4 KB lines).
  - The output is stored in f16 in an SBUF-mirror layout with 8 KB
    contiguous lines ([B_loc, 7, 2, 50, 4096]) and decoded on the host.
    A-half (partitions 0-49) and B-half (64-113) go out as separate
    DMAs that land on complementary SDMA engines.
  - Per-wave emission is software-pipelined (front = step1+logits of
    wave w+1 emitted before back = softmax+einsum of wave w) so the PE
    queue never stalls waiting for the exp() of the current wave.
"""

import numpy as np

import concourse.bacc as bacc
import concourse.bass as bass
import concourse.tile as tile
from concourse import mybir
from concourse.bass_utils import run_bass_kernel_spmd
from concourse.masks import make_identity

F32 = mybir.dt.float32
F16 = mybir.dt.float16
AX = mybir.AxisListType
AF = mybir.ActivationFunctionType

N_CORES = 8
B = 32
S = 100          # sentences per batch
L = 25           # words per sentence
C = 50           # classes
H = 512          # hidden
B_LOC = B // N_CORES          # batches per core
WAVE_S = 16                   # sentences per full wave
N_WAVES = 7                   # 6 full waves + 1 short wave (4 sentences)

_CACHE = {}
LAST_RESULT = None


def build_nc():
    nc = bacc.Bacc(trn_type="TRN2", target_bir_lowering=False, debug=False,
                   num_swdge_queues=2)
    xt_d = nc.declare_dram_parameter("xT", [B_LOC, H, S * L], F16, isOutput=False)
    xp_d = nc.declare_dram_parameter("xp", [B_LOC, N_WAVES, 128, 2048], F16, isOutput=False)
    wt_d = nc.declare_dram_parameter("WT", [H, H], F16, isOutput=False)
    b_d = nc.declare_dram_parameter("b", [H], F32, isOutput=False)
    ct_d = nc.declare_dram_parameter("CT", [128, 4 * C], F16, isOutput=False)
    o_d = nc.declare_dram_parameter("out", [B_LOC, N_WAVES, 2, C, 4096], F16, isOutput=True)

    q_load = [nc.sync, nc.scalar]
    q_store = [nc.gpsimd, nc.sync, nc.scalar]
    qctr = [0, 0]

    with tile.TileContext(nc) as tc:
        with tc.tile_pool(name="sb", bufs=1) as sb, \
             tc.tile_pool(name="consts", bufs=1) as consts, \
             tc.tile_pool(name="ps", bufs=1, space="PSUM") as ps:

            # ---------------- one-time setup ----------------
            ident_f = consts.tile([128, 128], F32)
            make_identity(nc, ident_f)
            ident_h = consts.tile([128, 128], F16)
            nc.vector.tensor_copy(ident_h, ident_f)

            b_sb = consts.tile([128, 4], F32)
            nc.sync.dma_start(out=b_sb, in_=b_d.rearrange("(k p) -> p k", p=128))

            # W^T tiles: w_t[i] is [i-part 128, o 512] (f16), host-transposed
            w_t = []
            for i in range(4):
                wt = consts.tile([128, 512], F16, name=f"w_t{i}")
                nc.scalar.dma_start(out=wt, in_=wt_d[i * 128:(i + 1) * 128, :])
                w_t.append(wt)

            # C^T tile: [i-part 128, o_chunk 4, c 50] (f16), host-arranged
            c_t = consts.tile([128, 4 * C], F16)
            nc.sync.dma_start(out=c_t, in_=ct_d[:, :])

            # x^T tiles: 4 chunks of [128, 2500] per batch, double-buffered
            def load_xt(bi):
                tiles = []
                for i in range(4):
                    xt = sb.tile([128, S * L], F16, tag="xt", bufs=8,
                                 name=f"xt{bi}_{i}")
                    q_load[(qctr[0] + i) % 2].dma_start(
                        out=xt, in_=xt_d[bi, i * 128:(i + 1) * 128, :])
                    tiles.append(xt)
                qctr[0] += 1
                return tiles

            xt_cur = [None, None]
            xt_cur[0] = load_xt(0)

            # ---------------- main loop (software-pipelined) ----------------
            def emit_front(bi, wv):
                xts = xt_cur[bi % 2]
                ns = WAVE_S if wv < 6 else 4
                G = ns // 4                   # sentence groups (4 or 1)
                TCOL = ns * L                 # dense t cols (400 or 100)
                t0 = wv * WAVE_S * L          # dense t offset

                # -- load packed natural x for the einsum --
                xp_t = sb.tile([128, 2048], F16, tag="xp", bufs=3,
                               name=f"xp{bi}_{wv}")
                q_load[qctr[1] % 2].dma_start(
                    out=xp_t[:, :512 * G], in_=xp_d[bi, wv, :, :512 * G])
                qctr[1] += 1

                # -- step 1: h^T[o] = tanh(W @ x^T + b), dense t --
                h = []
                for o in range(4):
                    ph = ps.tile([128, 512], F32, tag="ph", bufs=2,
                                 name=f"ph{bi}_{wv}_{o}")
                    for i in range(4):
                        nc.tensor.matmul(
                            ph[:, :TCOL],
                            w_t[i][:, o * 128:(o + 1) * 128],
                            xts[i][:, t0:t0 + TCOL],
                            start=(i == 0), stop=(i == 3),
                        )
                    ht = sb.tile([128, 512], F16, tag="h", bufs=8,
                                 name=f"h{bi}_{wv}_{o}")
                    nc.scalar.activation(
                        out=ht[:, :TCOL], in_=ph[:, :TCOL],
                        func=AF.Tanh, bias=b_sb[:, o:o + 1], scale=1.0,
                    )
                    h.append(ht)

                # -- step 2: logits[c, t] dense (accumulate over o) --
                pl = ps.tile([64, 512], F32, tag="pl", bufs=2,
                             name=f"pl{bi}_{wv}")
                for o in range(4):
                    nc.tensor.matmul(
                        pl[:C, :TCOL], c_t[:, o * C:(o + 1) * C],
                        h[o][:, :TCOL],
                        start=(o == 0), stop=(o == 3),
                    )
                return bi, wv, xp_t, pl

            def emit_back(state):
                bi, wv, xp_t, pl = state
                ns = WAVE_S if wv < 6 else 4
                G = ns // 4

                plb = pl[:C, :]
                pl_v = bass.AP(tensor=pl.tensor, offset=plb.offset,
                               ap=[plb.ap[0], [L, ns], [1, L]])

                # -- m = max over words (dense strided windows) --
                m = sb.tile([64, WAVE_S], F32, tag="m", bufs=3,
                            name=f"m{bi}_{wv}")
                nc.vector.reduce_max(out=m[:C, :ns], in_=pl_v, axis=AX.X)

                # -- e = exp(logits - m), written into 32-padded layout --
                epre = sb.tile([64, 512], F16, tag="epre", bufs=3,
                               name=f"epre{bi}_{wv}")
                e_sb = sb.tile([64, 512], F16, tag="e", bufs=3,
                               name=f"e{bi}_{wv}")
                eb = epre[:C, :]
                ep_v = bass.AP(tensor=epre.tensor, offset=eb.offset,
                               ap=[eb.ap[0], [32, ns], [1, L]])
                esb = e_sb[:C, :]
                e_v = bass.AP(tensor=e_sb.tensor, offset=esb.offset,
                              ap=[esb.ap[0], [32, ns], [1, L]])
                mb = m[:C, :]
                m_v = bass.AP(tensor=m.tensor, offset=mb.offset,
                              ap=[mb.ap[0], [1, ns], [0, L]])
                nc.vector.tensor_sub(ep_v, pl_v, m_v)
                nc.scalar.activation(out=e_v, in_=ep_v, func=AF.Exp)

                # -- e^T via f16 PE transposes -> one merged attn tile --
                pet = ps.tile([128, 256], F16, tag="pet", bufs=1,
                              name=f"pet{bi}_{wv}")
                for g in range(G):
                    nc.tensor.transpose(
                        pet[:, 64 * g:64 * g + C],
                        e_sb[:C, 128 * g:128 * (g + 1)],
                        ident_h[:C, :C],
                    )
                attn = sb.tile([128, 256], F16, tag="attn", bufs=3,
                               name=f"attn{bi}_{wv}")
                nc.vector.tensor_copy(attn.bitcast(F32), pet.bitcast(F32))

                # batched normalization: z[g] = 1/sum_c, attn *= z
                att_v = bass.AP(tensor=attn.tensor, offset=attn.offset,
                                ap=[attn.ap[0], [64, G], [1, C]])
                z = sb.tile([128, 4], F32, tag="z", bufs=3,
                            name=f"z{bi}_{wv}")
                nc.vector.reduce_sum(out=z[:, :G], in_=att_v, axis=AX.X)
                nc.vector.reciprocal(out=z[:, :G], in_=z[:, :G])
                z_v = bass.AP(tensor=z.tensor, offset=z.offset,
                              ap=[z.ap[0], [1, G], [0, C]])
                nc.vector.tensor_mul(att_v, att_v, z_v)

                # -- einsum: out[c, o] per sentence; 4xK 2xM packed f16 --
                osb = sb.tile([128, 4096], F16, tag="osb", bufs=3,
                              name=f"osb{bi}_{wv}")
                n_pairs = max(1, G // 2)
                gl_count = 2 if G >= 2 else 1
                nrow = 64 * (gl_count - 1) + C
                ci = 0
                for pi in range(n_pairs):
                    for jj in range(4):
                        po = ps.tile([128, 512], F32, tag="po", bufs=3,
                                     name=f"po{bi}_{wv}_{pi}_{jj}")
                        for gl in range(gl_count):
                            g = 2 * pi + gl
                            nc.tensor.matmul(
                                po[64 * gl:64 * gl + C, :],
                                attn[32 * jj:32 * jj + L, 64 * g:64 * g + C],
                                xp_t[32 * jj:32 * jj + L,
                                     512 * g:512 * (g + 1)],
                                start=True, stop=True,
                                tile_position=(32 * jj, 64 * gl),
                            )
                        dstc = osb[:nrow, 512 * (4 * pi + jj):
                                   512 * (4 * pi + jj + 1)]
                        if ci % 2 == 0:
                            nc.vector.tensor_copy(dstc, po[:nrow, :])
                        else:
                            nc.scalar.copy(dstc, po[:nrow, :])
                        ci += 1

                # -- stores: A half (partitions 0-49), B half (64-113) --
                ncols = 2048 * n_pairs
                si = (bi * N_WAVES + wv)
                q_store[si % 3].dma_start(
                    out=o_d[bi, wv, 0, :, :ncols], in_=osb[:C, :ncols])
                if gl_count == 2:
                    q_store[(si + 1) % 3].dma_start(
                        out=o_d[bi, wv, 1, :, :ncols],
                        in_=osb[64:64 + C, :ncols])

            pending = []
            for bi in range(B_LOC):
                for wv in range(N_WAVES):
                    if wv == 5 and bi + 1 < B_LOC:
                        xt_cur[(bi + 1) % 2] = load_xt(bi + 1)
                    pending.append(emit_front(bi, wv))
                    if len(pending) >= 2:
                        emit_back(pending.pop(0))
            while pending:
                emit_back(pending.pop(0))

    nc.compile()
    return nc


def _host_prep(x16):
    """Build per-core xT (host-transposed) and xp (packed SBUF image)."""
    xT = np.ascontiguousarray(x16.transpose(0, 2, 1))           # [B_LOC, H, T]
    xs = x16.reshape(B_LOC, S, L, H)
    xp = np.zeros((B_LOC, N_WAVES, 128, 2048), dtype=np.float16)
    for g in range(4):
        for jj in range(4):
            # full waves: sentence 16*wv + 4*g + jj
            sidx = np.arange(6) * WAVE_S + 4 * g + jj
            xp[:, :6, 32 * jj:32 * jj + L, 512 * g:512 * (g + 1)] = \
                xs[:, sidx, :, :]
    for jj in range(4):
        xp[:, 6, 32 * jj:32 * jj + L, 0:512] = xs[:, 96 + jj, :, :]
    return xT, xp


def _host_decode(raw):
    """Decode the SBUF-mirror f16 output into [B_LOC, S, C, H] f32."""
    out = np.empty((B_LOC, S, C, H), dtype=np.float32)
    v = raw.reshape(B_LOC, N_WAVES, 2, C, 8, H)
    for wv in range(6):
        for k in range(8):
            s0 = WAVE_S * wv + 8 * (k // 4)
            out[:, s0 + (k % 4)] = v[:, wv, 0, :, k]
            out[:, s0 + 4 + (k % 4)] = v[:, wv, 1, :, k]
    for k in range(4):
        out[:, 96 + k] = v[:, 6, 0, :, k]
    return out


def kernel(**inputs):
    global LAST_RESULT
    if "nc" not in _CACHE:
        _CACHE["nc"] = build_nc()
    nc = _CACHE["nc"]

    x = np.asarray(inputs["input_tensor"], dtype=np.float32).astype(np.float16)
    w = np.asarray(inputs["W"], dtype=np.float32).astype(np.float16)
    bb = np.ascontiguousarray(inputs["b"], dtype=np.float32)
    cv = np.asarray(inputs["context_vector"], dtype=np.float32).astype(np.float16)

    wt_h = np.ascontiguousarray(w.T)                            # [I, O]
    ct_h = np.ascontiguousarray(
        cv.T.reshape(4, 128, C).transpose(1, 0, 2).reshape(128, 4 * C))

    in_maps = []
    for ci in range(N_CORES):
        xT, xp = _host_prep(x[ci * B_LOC:(ci + 1) * B_LOC])
        in_maps.append({
            "xT": xT,
            "xp": xp,
            "WT": wt_h,
            "b": bb,
            "CT": ct_h,
        })
    res = run_bass_kernel_spmd(nc, in_maps, core_ids=list(range(N_CORES)))
    LAST_RESULT = res
    out = np.empty((B, S, C, H), dtype=np.float32)
    for ci in range(N_CORES):
        out[ci * B_LOC:(ci + 1) * B_LOC] = _host_decode(res.results[ci]["out"])
    return out


# revision 4
# speedup vs baseline: 1.4819x; 1.1202x over previous
"""AttentionPerLabelWordLevel Trainium2 kernel (8-core SPMD, batch-sharded).

Reference computation (per batch b):
  h = tanh(x @ W.T + b)                      # [T, H]
  logits = h @ C.T                           # [S, L, C]
  m = max_L(logits)                          # [S, 1, C]
  attn = softmax_C(logits - m)               # [S, L, C]
  out[s, c, :] = sum_l attn[s, l, c] * x[s, l, :]   # [S, C, H]

Shapes: B=32, T=2500 (S=100 sentences x L=25 words), H=512, C=50.
Sharding: data-parallel over batch, 4 batches per core.

v3 design:
  - x^T host-pretransposed ([B_loc, H, T] f16), x natural host-packed
    into the SBUF einsum image; W^T / C^T host-arranged; f16 output in
    an SBUF-mirror layout with fat contiguous lines, decoded on host.
  - step 1/2 run on a DENSE t axis (no 25->32 padding).
  - 4-stage software pipeline, one wave per stage per iteration:
      F(w)   step1 matmuls + tanh + logits          [PE, ACT]
      A(w)   softmax-early: max, sub, exp           [DVE, ACT]
      B(w)   e^T transpose, attn copy, z-norm       [PE, DVE, GpSimd]
      C(w)   einsum + PSUM->SBUF copies + stores    [PE, DVE/ACT, DMA]
    Iteration i emits A(w[i-1]), C(w[i-3]) interleaved with F(w[i]),
    then B(w[i-2]) - every op's producers ran >= 1 iteration earlier,
    so no engine FIFO ever head-of-line blocks and the PE stays warm.
  - PSUM: ph x2 banks, pl x1, shared {po,pet} ring x5 = 8 banks.
  - DMA: loads on the Sync HWDGE ring; A/B output halves on the two
    GpSimd SWDGE queues (complementary SDMA engines, run concurrently).
    Scalar engine issues no DMA (it is compute-saturated).
  - xp loads skip the 7 zero pad rows per 32-row block (4 DMAs of 25
    partitions instead of 1 of 128: -2.3 MB HBM traffic per core).
"""

import numpy as np

import concourse.bacc as bacc
import concourse.bass as bass
import concourse.tile as tile
from concourse import mybir
from concourse.bass_utils import run_bass_kernel_spmd
from concourse.masks import make_identity

F32 = mybir.dt.float32
F16 = mybir.dt.float16
AX = mybir.AxisListType
AF = mybir.ActivationFunctionType

N_CORES = 8
B = 32
S = 100          # sentences per batch
L = 25           # words per sentence
C = 50           # classes
H = 512          # hidden
B_LOC = B // N_CORES          # batches per core
WAVE_S = 16                   # sentences per full wave
N_WAVES = 7                   # 6 full waves + 1 short wave (4 sentences)

_CACHE = {}
LAST_RESULT = None


def build_nc():
    nc = bacc.Bacc(trn_type="TRN2", target_bir_lowering=False, debug=False,
                   num_swdge_queues=2)
    xt_d = nc.declare_dram_parameter("xT", [B_LOC, H, S * L], F16, isOutput=False)
    xp_d = nc.declare_dram_parameter("xp", [B_LOC, N_WAVES, 128, 2048], F16, isOutput=False)
    wt_d = nc.declare_dram_parameter("WT", [H, H], F16, isOutput=False)
    b_d = nc.declare_dram_parameter("b", [H], F32, isOutput=False)
    ct_d = nc.declare_dram_parameter("CT", [128, 4 * C], F16, isOutput=False)
    o_d = nc.declare_dram_parameter("out", [B_LOC, N_WAVES, 2, C, 4096], F16, isOutput=True)

    with tile.TileContext(nc) as tc:
        with tc.tile_pool(name="sb", bufs=1) as sb, \
             tc.tile_pool(name="consts", bufs=1) as consts, \
             tc.tile_pool(name="ps", bufs=1, space="PSUM") as ps:

            # ---------------- one-time setup ----------------
            ident_f = consts.tile([128, 128], F32)
            make_identity(nc, ident_f)
            ident_h = consts.tile([128, 128], F16)
            nc.vector.tensor_copy(ident_h, ident_f)

            b_sb = consts.tile([128, 4], F32)
            nc.sync.dma_start(out=b_sb, in_=b_d.rearrange("(k p) -> p k", p=128))

            w_t = []
            for i in range(4):
                wt = consts.tile([128, 512], F16, name=f"w_t{i}")
                nc.sync.dma_start(out=wt, in_=wt_d[i * 128:(i + 1) * 128, :])
                w_t.append(wt)

            c_t = consts.tile([128, 4 * C], F16)
            nc.sync.dma_start(out=c_t, in_=ct_d[:, :])

            def load_xt(bi):
                tiles = []
                for i in range(4):
                    xt = sb.tile([128, S * L], F16, tag="xt", bufs=8,
                                 name=f"xt{bi}_{i}")
                    nc.sync.dma_start(out=xt, in_=xt_d[bi, i * 128:(i + 1) * 128, :])
                    tiles.append(xt)
                return tiles

            xt_cur = [None, None]
            xt_cur[0] = load_xt(0)

            waves = [(bi, wv) for bi in range(B_LOC) for wv in range(N_WAVES)]
            NW = len(waves)

            state = {}   # wave index -> per-stage tiles

            def wave_dims(wi):
                bi, wv = waves[wi]
                ns = WAVE_S if wv < 6 else 4
                return bi, wv, ns, ns // 4, ns * L

            def emit_load(wi):
                bi, wv, ns, G, TCOL = wave_dims(wi)
                xp_t = sb.tile([128, 2048], F16, tag="xp", bufs=4,
                               name=f"xp{bi}_{wv}")
                for jj in range(4):
                    nc.sync.dma_start(
                        out=xp_t[32 * jj:32 * jj + L, :512 * G],
                        in_=xp_d[bi, wv, 32 * jj:32 * jj + L, :512 * G])
                state[wi] = {"xp": xp_t}

            def emit_F(wi):
                bi, wv, ns, G, TCOL = wave_dims(wi)
                xts = xt_cur[bi % 2]
                t0 = wv * WAVE_S * L
                st = state[wi]

                # step 1: h^T[o] = tanh(W @ x^T + b), dense t
                h = []
                for o in range(4):
                    ph = ps.tile([128, 512], F32, tag="ph", bufs=2,
                                 name=f"ph{bi}_{wv}_{o}")
                    for i in range(4):
                        nc.tensor.matmul(
                            ph[:, :TCOL],
                            w_t[i][:, o * 128:(o + 1) * 128],
                            xts[i][:, t0:t0 + TCOL],
                            start=(i == 0), stop=(i == 3),
                        )
                    ht = sb.tile([128, 512], F16, tag="h", bufs=8,
                                 name=f"h{bi}_{wv}_{o}")
                    nc.scalar.activation(
                        out=ht[:, :TCOL], in_=ph[:, :TCOL],
                        func=AF.Tanh, bias=b_sb[:, o:o + 1], scale=1.0,
                    )
                    h.append(ht)

                # step 2: logits[c, t] dense (accumulate over o)
                pl = ps.tile([64, 512], F32, tag="pl", bufs=1,
                             name=f"pl{bi}_{wv}")
                for o in range(4):
                    nc.tensor.matmul(
                        pl[:C, :TCOL], c_t[:, o * C:(o + 1) * C],
                        h[o][:, :TCOL],
                        start=(o == 0), stop=(o == 3),
                    )
                st["pl"] = pl

            def emit_A(wi):
                bi, wv, ns, G, TCOL = wave_dims(wi)
                st = state[wi]
                pl = st["pl"]

                plb = pl[:C, :]
                pl_v = bass.AP(tensor=pl.tensor, offset=plb.offset,
                               ap=[plb.ap[0], [L, ns], [1, L]])
                m = sb.tile([64, WAVE_S], F32, tag="m", bufs=3,
                            name=f"m{bi}_{wv}")
                nc.vector.reduce_max(out=m[:C, :ns], in_=pl_v, axis=AX.X)

                epre = sb.tile([64, 512], F16, tag="epre", bufs=3,
                               name=f"epre{bi}_{wv}")
                e_sb = sb.tile([64, 512], F16, tag="e", bufs=3,
                               name=f"e{bi}_{wv}")
                eb = epre[:C, :]
                ep_v = bass.AP(tensor=epre.tensor, offset=eb.offset,
                               ap=[eb.ap[0], [32, ns], [1, L]])
                esb = e_sb[:C, :]
                e_v = bass.AP(tensor=e_sb.tensor, offset=esb.offset,
                              ap=[esb.ap[0], [32, ns], [1, L]])
                mb = m[:C, :]
                m_v = bass.AP(tensor=m.tensor, offset=mb.offset,
                              ap=[mb.ap[0], [1, ns], [0, L]])
                nc.vector.tensor_sub(ep_v, pl_v, m_v)
                nc.scalar.activation(out=e_v, in_=ep_v, func=AF.Exp)
                st["e"] = e_sb

            def emit_B(wi):
                bi, wv, ns, G, TCOL = wave_dims(wi)
                st = state[wi]
                e_sb = st["e"]

                pet = ps.tile([128, 256], F16, tag="pp", bufs=5,
                              name=f"pet{bi}_{wv}")
                for g in range(G):
                    nc.tensor.transpose(
                        pet[:, 64 * g:64 * g + C],
                        e_sb[:C, 128 * g:128 * (g + 1)],
                        ident_h[:C, :C],
                    )
                attn = sb.tile([128, 256], F16, tag="attn", bufs=3,
                               name=f"attn{bi}_{wv}")
                nc.vector.tensor_copy(attn.bitcast(F32), pet.bitcast(F32))

                att_v = bass.AP(tensor=attn.tensor, offset=attn.offset,
                                ap=[attn.ap[0], [64, G], [1, C]])
                z = sb.tile([128, 4], F32, tag="z", bufs=3,
                            name=f"z{bi}_{wv}")
                nc.vector.reduce_sum(out=z[:, :G], in_=att_v, axis=AX.X)
                nc.vector.reciprocal(out=z[:, :G], in_=z[:, :G])
                z_v = bass.AP(tensor=z.tensor, offset=z.offset,
                              ap=[z.ap[0], [1, G], [0, C]])
                nc.gpsimd.tensor_mul(att_v, att_v, z_v)
                st["attn"] = attn

            def emit_C_mms(wi, po_range):
                """Einsum matmuls + psum->sbuf copies for po indices in
                po_range (a po = one (pair, jj): up to 2 tile-packed MMs)."""
                bi, wv, ns, G, TCOL = wave_dims(wi)
                st = state[wi]
                attn = st["attn"]
                xp_t = st["xp"]
                gl_count = 2 if G >= 2 else 1
                nrow = 64 * (gl_count - 1) + C
                if "osb" not in st:
                    st["osb"] = sb.tile([128, 4096], F16, tag="osb", bufs=3,
                                        name=f"osb{bi}_{wv}")
                osb = st["osb"]
                n_pairs = max(1, G // 2)
                for k in po_range:
                    if k >= 4 * n_pairs:
                        continue
                    pi, jj = k // 4, k % 4
                    po = ps.tile([128, 512], F32, tag="pp", bufs=5,
                                 name=f"po{bi}_{wv}_{pi}_{jj}")
                    for gl in range(gl_count):
                        g = 2 * pi + gl
                        nc.tensor.matmul(
                            po[64 * gl:64 * gl + C, :],
                            attn[32 * jj:32 * jj + L, 64 * g:64 * g + C],
                            xp_t[32 * jj:32 * jj + L, 512 * g:512 * (g + 1)],
                            start=True, stop=True,
                            tile_position=(32 * jj, 64 * gl),
                        )
                    dstc = osb[:nrow, 512 * k:512 * (k + 1)]
                    if k % 3 == 2:
                        nc.scalar.copy(dstc, po[:nrow, :])
                    else:
                        nc.vector.tensor_copy(dstc, po[:nrow, :])

            def emit_store(wi):
                bi, wv, ns, G, TCOL = wave_dims(wi)
                st = state[wi]
                osb = st["osb"]
                n_pairs = max(1, G // 2)
                ncols = 2048 * n_pairs
                nc.gpsimd.dma_start(
                    out=o_d[bi, wv, 0, :, :ncols], in_=osb[:C, :ncols])
                if G >= 2:
                    nc.gpsimd.dma_start(
                        out=o_d[bi, wv, 1, :, :ncols],
                        in_=osb[64:64 + C, :ncols])
                del state[wi]

            # ---------------- pipelined emission ----------------
            # iteration i: A(i-1), C(i-3) split around F(i), B(i-2)
            for i in range(NW + 3):
                if i < NW:
                    bi, wv = waves[i]
                    if wv == 4 and bi + 1 < B_LOC:
                        xt_cur[(bi + 1) % 2] = load_xt(bi + 1)
                    if i == 0:
                        emit_load(0)
                if i - 1 >= 0 and i - 1 < NW:
                    emit_A(i - 1)
                if i - 3 >= 0:
                    emit_C_mms(i - 3, range(0, 5))
                if i < NW:
                    emit_F(i)
                    if i + 1 < NW:
                        emit_load(i + 1)
                if i - 3 >= 0:
                    emit_C_mms(i - 3, range(5, 8))
                    emit_store(i - 3)
                if i - 2 >= 0 and i - 2 < NW:
                    emit_B(i - 2)

    nc.compile()
    return nc


def _host_prep(x16):
    """Build per-core xT (host-transposed) and xp (packed SBUF image)."""
    xT = np.ascontiguousarray(x16.transpose(0, 2, 1))           # [B_LOC, H, T]
    xs = x16.reshape(B_LOC, S, L, H)
    xp = np.zeros((B_LOC, N_WAVES, 128, 2048), dtype=np.float16)
    for g in range(4):
        for jj in range(4):
            sidx = np.arange(6) * WAVE_S + 4 * g + jj
            xp[:, :6, 32 * jj:32 * jj + L, 512 * g:512 * (g + 1)] = \
                xs[:, sidx, :, :]
    for jj in range(4):
        xp[:, 6, 32 * jj:32 * jj + L, 0:512] = xs[:, 96 + jj, :, :]
    return xT, xp


def _host_decode(raw):
    """Decode the SBUF-mirror f16 output into [B_LOC, S, C, H] f32."""
    out = np.empty((B_LOC, S, C, H), dtype=np.float32)
    v = raw.reshape(B_LOC, N_WAVES, 2, C, 8, H)
    for wv in range(6):
        for k in range(8):
            s0 = WAVE_S * wv + 8 * (k // 4)
            out[:, s0 + (k % 4)] = v[:, wv, 0, :, k]
            out[:, s0 + 4 + (k % 4)] = v[:, wv, 1, :, k]
    for k in range(4):
        out[:, 96 + k] = v[:, 6, 0, :, k]
    return out


def kernel(**inputs):
    global LAST_RESULT
    if "nc" not in _CACHE:
        _CACHE["nc"] = build_nc()
    nc = _CACHE["nc"]

    x = np.asarray(inputs["input_tensor"], dtype=np.float32).astype(np.float16)
    w = np.asarray(inputs["W"], dtype=np.float32).astype(np.float16)
    bb = np.ascontiguousarray(inputs["b"], dtype=np.float32)
    cv = np.asarray(inputs["context_vector"], dtype=np.float32).astype(np.float16)

    wt_h = np.ascontiguousarray(w.T)                            # [I, O]
    ct_h = np.ascontiguousarray(
        cv.T.reshape(4, 128, C).transpose(1, 0, 2).reshape(128, 4 * C))

    in_maps = []
    for ci in range(N_CORES):
        xT, xp = _host_prep(x[ci * B_LOC:(ci + 1) * B_LOC])
        in_maps.append({
            "xT": xT,
            "xp": xp,
            "WT": wt_h,
            "b": bb,
            "CT": ct_h,
        })
    res = run_bass_kernel_spmd(nc, in_maps, core_ids=list(range(N_CORES)))
    LAST_RESULT = res
    out = np.empty((B, S, C, H), dtype=np.float32)
    for ci in range(N_CORES):
        out[ci * B_LOC:(ci + 1) * B_LOC] = _host_decode(res.results[ci]["out"])
    return out
